# revision 1
# baseline (speedup 1.0000x reference)
"""v4: monolithic bf16 attention on 8 trn2 NeuronCores (no collectives).

Core c handles batch c//2, query half c%2, loading the batch's full K and V.
All inputs are cast fp32->bf16 during the SWDGE DMA load using contiguous
per-partition access (each partition holds consecutive sequence rows; the
resulting sequence permutation cancels in softmax/attnV for K/V and is
undone in the output DMA for Q). PE transposes and matmuls run bf16 at
1 cycle/row with fp32 PSUM accumulation. Scores are computed in the [k, q]
layout (softmax free-dim never reduced: ones-column in the attnV stationary
yields row-sums; normalization deferred to the output epilogue).
"""

import sys

if "/opt/trn_rl_repo" not in sys.path:
    sys.path.insert(0, "/opt/trn_rl_repo")

import numpy as np

N, L, H, D = 4, 2048, 1024, 64
QSH = L // 2
NCORES = 8
HC = H // 128
NRNG = L // 512


def build_bass():
    import concourse.bass as bass
    import concourse.mybir as mybir
    from concourse import bacc
    from concourse.masks import make_identity
    from concourse.tile import TileContext

    f32 = mybir.dt.float32
    bf16 = mybir.dt.bfloat16
    AF = mybir.ActivationFunctionType

    nc = bacc.Bacc("TRN2", target_bir_lowering=False, debug=False)
    q_d = nc.dram_tensor("q", [QSH, H], f32, kind="ExternalInput").ap()
    k_d = nc.dram_tensor("k", [L, H], f32, kind="ExternalInput").ap()
    v_d = nc.dram_tensor("v", [L, H], f32, kind="ExternalInput").ap()
    wq_d = nc.dram_tensor("wq", [H, D], f32, kind="ExternalInput").ap()
    wk_d = nc.dram_tensor("wk", [H, D], f32, kind="ExternalInput").ap()
    wv_d = nc.dram_tensor("wv", [H, D], f32, kind="ExternalInput").ap()
    bq_d = nc.dram_tensor("bq8", [D, 1], f32, kind="ExternalInput").ap()
    bk_d = nc.dram_tensor("bk", [D, 1], f32, kind="ExternalInput").ap()
    bv_d = nc.dram_tensor("bv", [D, 1], f32, kind="ExternalInput").ap()
    out_d = nc.dram_tensor("out", [QSH, D], f32, kind="ExternalOutput").ap()

    with TileContext(nc) as tc:
        with (
            tc.tile_pool(name="const", bufs=1) as const_pool,
            tc.tile_pool(name="w", bufs=1) as w_pool,
            tc.tile_pool(name="qnat", bufs=1) as qnat_pool,
            tc.tile_pool(name="qT", bufs=1) as qT_pool,
            tc.tile_pool(name="nat", bufs=8) as nat_pool,
            tc.tile_pool(name="rT", bufs=4) as rT_pool,
            tc.tile_pool(name="proj", bufs=1) as proj_pool,
            tc.tile_pool(name="vp", bufs=1) as vp_pool,
            tc.tile_pool(name="exp", bufs=6) as exp_pool,
            tc.tile_pool(name="fin", bufs=1) as fin_pool,
            tc.tile_pool(name="tpb", bufs=2, space="PSUM") as tpb_psum,
            tc.tile_pool(name="pj", bufs=1, space="PSUM") as pj_psum,
            tc.tile_pool(name="sc", bufs=2, space="PSUM") as sc_psum,
            tc.tile_pool(name="acc", bufs=1, space="PSUM") as acc_psum,
        ):
            identb = const_pool.tile([128, 128], bf16)
            make_identity(nc, identb[:])
            identf = const_pool.tile([128, 128], f32, tag="identf")
            make_identity(nc, identf[:])
            ones_sb = const_pool.tile([128, 1], bf16, tag="ones")
            nc.vector.memset(ones_sb[:], 1.0)

            w_sb = {}
            for name, wd in (("wq", wq_d), ("wk", wk_d), ("wv", wv_d)):
                t = w_pool.tile([128, HC * D], bf16, tag=name, name=name)
                nc.gpsimd.dma_start(
                    out=t[:].rearrange("p (c d) -> p c d", c=HC),
                    in_=wd.rearrange("(c p) d -> p c d", p=128),
                )
                w_sb[name] = t
            bq_sb = const_pool.tile([D, 1], f32, tag="bq")
            bk_sb = const_pool.tile([D, 1], f32, tag="bk")
            bv_sb = const_pool.tile([D, 1], f32, tag="bv")
            nc.sync.dma_start(out=bq_sb[:], in_=bq_d[:])
            nc.sync.dma_start(out=bk_sb[:], in_=bk_d[:])
            nc.sync.dma_start(out=bv_sb[:], in_=bv_d[:])

            def transpose_range(dst, src, jcnt):
                for hp in range(HC // 2):
                    ps = tpb_psum.tile([128, 1024], bf16, tag="tpb", name="psb")
                    for half in range(2):
                        hc = hp * 2 + half
                        for s in range(jcnt):
                            nc.tensor.transpose(
                                ps[:, half * 512 + s * 128 : half * 512 + (s + 1) * 128],
                                src[:, s * H + hc * 128 : s * H + (hc + 1) * 128],
                                identb[:],
                            )
                    nc.vector.tensor_copy(dst[:, hp * 1024 : (hp + 1) * 1024], ps[:])

            # ---- Q ----
            q_nat = qnat_pool.tile([128, 8 * H], bf16)
            for ai in range(2):
                nc.gpsimd.dma_start(
                    out=q_nat[:, ai * 4 * H : (ai + 1) * 4 * H],
                    in_=q_d.rearrange("(p a j) h -> a p (j h)", a=2, j=4)[ai],
                )
            qT = [qT_pool.tile([128, QSH], bf16, tag=f"qT{h}", name=f"qT{h}")
                  for h in range(HC)]
            for hc in range(HC):
                ps = tpb_psum.tile([128, 1024], bf16, tag="tpb", name="psb")
                for qc in range(8):
                    nc.tensor.transpose(
                        ps[:, qc * 128 : (qc + 1) * 128],
                        q_nat[:, qc * H + hc * 128 : qc * H + (hc + 1) * 128],
                        identb[:],
                    )
                nc.vector.tensor_copy(qT[hc][:], ps[:])
            qprojT = proj_pool.tile([D, QSH], bf16, tag="qprojT")
            for qn in range(QSH // 512):
                ps = pj_psum.tile([D, 512], f32, tag="pj", name="pjq")
                for hc in range(HC):
                    nc.tensor.matmul(
                        ps[:],
                        w_sb["wq"][:, hc * D : (hc + 1) * D],
                        qT[hc][:, qn * 512 : (qn + 1) * 512],
                        start=(hc == 0), stop=(hc == HC - 1),
                    )
                nc.scalar.activation(
                    qprojT[:, qn * 512 : (qn + 1) * 512], ps[:],
                    AF.Identity, bias=bq_sb[:], scale=0.125,
                )

            # ---- V ----
            vprojT = proj_pool.tile([D, L], bf16, tag="vprojT")
            vp = vp_pool.tile([128, (L // 128) * 65], bf16, tag="vp")
            for rng in range(NRNG):
                v_nat = nat_pool.tile([128, 4 * H], bf16, tag="nat",
                                      name=f"vnat{rng}")
                nc.gpsimd.dma_start(
                    out=v_nat[:],
                    in_=v_d.rearrange("(r p j) h -> r p (j h)", p=128, j=4)[rng],
                )
                vT = rT_pool.tile([128, HC * 512], bf16, tag="rT",
                                  name=f"vT{rng}")
                transpose_range(vT, v_nat, 4)
                ps = pj_psum.tile([D, 512], f32, tag="pj", name="pjv")
                for hc in range(HC):
                    nc.tensor.matmul(
                        ps[:], w_sb["wv"][:, hc * D : (hc + 1) * D],
                        vT[:, hc * 512 : (hc + 1) * 512],
                        start=(hc == 0), stop=(hc == HC - 1),
                    )
                vs = vprojT[:, rng * 512 : (rng + 1) * 512]
                nc.scalar.activation(vs, ps[:], AF.Identity, bias=bv_sb[:])
                psv = tpb_psum.tile([128, 512], bf16, tag="tpv", name="psv",
                                    bufs=1)
                for s in range(4):
                    nc.tensor.transpose(
                        psv[:, s * 128 : s * 128 + D],
                        vs[:, s * 128 : (s + 1) * 128],
                        identb[0:D, 0:D],
                    )
                for s in range(4):
                    kc = rng * 4 + s
                    nc.vector.tensor_copy(
                        vp[:, kc * 65 : kc * 65 + 64],
                        psv[:, s * 128 : s * 128 + D],
                    )
                    nc.vector.tensor_copy(
                        vp[:, kc * 65 + 64 : kc * 65 + 65], ones_sb[:]
                    )

            # ---- K + scores + attnV ----
            kprojT = proj_pool.tile([D, L], bf16, tag="kprojT")
            outT_ps = acc_psum.tile([65, QSH], f32)
            for rng in range(NRNG):
                k_nat = nat_pool.tile([128, 4 * H], bf16, tag="nat",
                                      name=f"knat{rng}")
                nc.gpsimd.dma_start(
                    out=k_nat[:],
                    in_=k_d.rearrange("(r p j) h -> r p (j h)", p=128, j=4)[rng],
                )
                kT = rT_pool.tile([128, HC * 512], bf16, tag="rT",
                                  name=f"kT{rng}")
                transpose_range(kT, k_nat, 4)
                ps = pj_psum.tile([D, 512], f32, tag="pj", name="pjk")
                for hc in range(HC):
                    nc.tensor.matmul(
                        ps[:], w_sb["wk"][:, hc * D : (hc + 1) * D],
                        kT[:, hc * 512 : (hc + 1) * 512],
                        start=(hc == 0), stop=(hc == HC - 1),
                    )
                kslice = kprojT[:, rng * 512 : (rng + 1) * 512]
                nc.scalar.activation(kslice, ps[:], AF.Identity, bias=bk_sb[:])

                for s in range(4):
                    kc = rng * 4 + s
                    e = exp_pool.tile([128, QSH], bf16, tag="exp")
                    for qn in range(QSH // 512):
                        sc = sc_psum.tile([128, 512], f32, tag="sc")
                        nc.tensor.matmul(
                            sc[:],
                            kprojT[:, kc * 128 : (kc + 1) * 128],
                            qprojT[:, qn * 512 : (qn + 1) * 512],
                            start=True, stop=True,
                        )
                        nc.scalar.activation(
                            e[:, qn * 512 : (qn + 1) * 512], sc[:], AF.Exp
                        )
                    for qn in range(QSH // 512):
                        nc.tensor.matmul(
                            outT_ps[:, qn * 512 : (qn + 1) * 512],
                            vp[:, kc * 65 : (kc + 1) * 65],
                            e[:, qn * 512 : (qn + 1) * 512],
                            start=(kc == 0), stop=(kc == L // 128 - 1),
                            skip_group_check=True,
                        )

            # ---- finalize ----
            outT_sb = fin_pool.tile([65, QSH], f32, tag="outT")
            nc.vector.tensor_copy(outT_sb[:], outT_ps[:])
            out_sb = fin_pool.tile([128, 8 * D], f32, tag="out")
            for qc in range(QSH // 128):
                ps = pj_psum.tile([128, 128], f32, tag="pj", name="pjf")
                nc.tensor.transpose(
                    ps[:, 0:65],
                    outT_sb[:, qc * 128 : (qc + 1) * 128],
                    identf[0:65, 0:65],
                )
                recip = fin_pool.tile([128, 1], f32, tag="recip")
                nc.vector.reciprocal(recip[:], ps[:, 64:65])
                nc.vector.tensor_scalar_mul(
                    out_sb[:, qc * D : (qc + 1) * D], ps[:, 0:D], recip[:]
                )
            nc.sync.dma_start(
                out=out_d.rearrange("(p j) d -> p j d", j=8),
                in_=out_sb[:].rearrange("p (j d) -> p j d", j=8),
            )

    nc.compile()
    return nc


_NC_CACHE = None


def _get_nc():
    global _NC_CACHE
    if _NC_CACHE is None:
        _NC_CACHE = build_bass()
    return _NC_CACHE


def _make_in_maps(inputs):
    query = np.ascontiguousarray(np.asarray(inputs["query"], np.float32))
    key = np.ascontiguousarray(np.asarray(inputs["key"], np.float32))
    value = np.ascontiguousarray(np.asarray(inputs["value"], np.float32))
    wq = np.ascontiguousarray(np.asarray(inputs["Wq"], np.float32))
    wk = np.ascontiguousarray(np.asarray(inputs["Wk"], np.float32))
    wv = np.ascontiguousarray(np.asarray(inputs["Wv"], np.float32))
    bq8 = (np.asarray(inputs["bq"], np.float32) / 8.0).reshape(D, 1)
    bk = np.asarray(inputs["bk"], np.float32).reshape(D, 1).copy()
    bv = np.asarray(inputs["bv"], np.float32).reshape(D, 1).copy()
    in_maps = []
    for c in range(NCORES):
        b, half = divmod(c, 2)
        in_maps.append(
            {
                "q": query[b, half * QSH : (half + 1) * QSH],
                "k": key[b],
                "v": value[b],
                "wq": wq,
                "wk": wk,
                "wv": wv,
                "bq8": bq8,
                "bk": bk,
                "bv": bv,
            }
        )
    return in_maps


def kernel(query, key, value, Wq, bq, Wk, bk, Wv, bv):
    from concourse.bass_utils import run_bass_kernel_spmd

    in_maps = _make_in_maps(
        dict(query=query, key=key, value=value, Wq=Wq, bq=bq, Wk=Wk, bk=bk,
             Wv=Wv, bv=bv)
    )
    nc = _get_nc()
    try:
        res = run_bass_kernel_spmd(nc, in_maps, list(range(NCORES)))
    except Exception:
        res = run_bass_kernel_spmd(nc, in_maps, list(range(NCORES)))
    out = np.empty((N, L, D), np.float32)
    for c in range(NCORES):
        b, half = divmod(c, 2)
        out[b, half * QSH : (half + 1) * QSH] = res.results[c]["out"]
    return out



# revision 3
# speedup vs baseline: 1.4455x; 1.4455x over previous
"""v5: bf16 attention on 8 trn2 NeuronCores, host-transposed inputs.

Core c handles batch c//2, query half c%2, loading the batch's full K and V.
Host pre-casts q/k/v to bf16 and pre-transposes them to [H, L] layout so all
device DMA loads are linear with 2KB+ descriptors and the PE never transposes
the 1024-dim inputs (only the tiny 64-dim vproj and the output epilogue).
Scores are computed in the [k, q] layout (softmax free-dim never reduced:
ones-column in the attnV stationary yields row-sums; normalization deferred
to the output epilogue).
"""

import sys

if "/opt/trn_rl_repo" not in sys.path:
    sys.path.insert(0, "/opt/trn_rl_repo")

import numpy as np

N, L, H, D = 4, 2048, 1024, 64
QSH = L // 2
NCORES = 8
HC = H // 128
NRNG = L // 512


def build_bass():
    import concourse.bass as bass
    import concourse.mybir as mybir
    from concourse import bacc
    from concourse.masks import make_identity
    from concourse.tile import TileContext

    f32 = mybir.dt.float32
    bf16 = mybir.dt.bfloat16
    AF = mybir.ActivationFunctionType

    nc = bacc.Bacc("TRN2", target_bir_lowering=False, debug=False)
    # host-transposed, bf16: qT [H, QSH], kT/vT [H, L]
    qT_d = nc.dram_tensor("qT", [H, QSH], bf16, kind="ExternalInput").ap()
    kT_d = nc.dram_tensor("kT", [H, L], bf16, kind="ExternalInput").ap()
    vT_d = nc.dram_tensor("vT", [H, L], bf16, kind="ExternalInput").ap()
    wq_d = nc.dram_tensor("wq", [H, D], bf16, kind="ExternalInput").ap()
    wk_d = nc.dram_tensor("wk", [H, D], bf16, kind="ExternalInput").ap()
    wv_d = nc.dram_tensor("wv", [H, D], bf16, kind="ExternalInput").ap()
    bq_d = nc.dram_tensor("bq8", [D, 1], f32, kind="ExternalInput").ap()
    bk_d = nc.dram_tensor("bk", [D, 1], f32, kind="ExternalInput").ap()
    bv_d = nc.dram_tensor("bv", [D, 1], f32, kind="ExternalInput").ap()
    out_d = nc.dram_tensor("out", [QSH, D], f32, kind="ExternalOutput").ap()

    with TileContext(nc) as tc:
        with (
            tc.tile_pool(name="const", bufs=1) as const_pool,
            tc.tile_pool(name="w", bufs=1) as w_pool,
            tc.tile_pool(name="qT", bufs=1) as qT_pool,
            tc.tile_pool(name="kv", bufs=4) as kv_pool,
            tc.tile_pool(name="proj", bufs=1) as proj_pool,
            tc.tile_pool(name="vp", bufs=1) as vp_pool,
            tc.tile_pool(name="exp", bufs=6) as exp_pool,
            tc.tile_pool(name="fin", bufs=1) as fin_pool,
            tc.tile_pool(name="pj", bufs=2, space="PSUM") as pj_psum,
            tc.tile_pool(name="sc", bufs=2, space="PSUM") as sc_psum,
            tc.tile_pool(name="psv", bufs=1, space="PSUM") as psv_psum,
            tc.tile_pool(name="acc", bufs=1, space="PSUM") as acc_psum,
        ):
            identb = const_pool.tile([128, 128], bf16)
            make_identity(nc, identb[:])
            identf = const_pool.tile([128, 128], f32, tag="identf")
            make_identity(nc, identf[:])
            ones_sb = const_pool.tile([128, 1], bf16, tag="ones")
            nc.vector.memset(ones_sb[:], 1.0)

            w_sb = {}
            for name, wd in (("wq", wq_d), ("wk", wk_d), ("wv", wv_d)):
                t = w_pool.tile([128, HC * D], bf16, tag=name, name=name)
                nc.gpsimd.dma_start(
                    out=t[:].rearrange("p (c d) -> p c d", c=HC),
                    in_=wd.rearrange("(c p) d -> p c d", p=128),
                )
                w_sb[name] = t
            bq_sb = const_pool.tile([D, 1], f32, tag="bq")
            bk_sb = const_pool.tile([D, 1], f32, tag="bk")
            bv_sb = const_pool.tile([D, 1], f32, tag="bv")
            nc.gpsimd.dma_start(out=bq_sb[:], in_=bq_d[:])
            nc.gpsimd.dma_start(out=bk_sb[:], in_=bk_d[:])
            nc.gpsimd.dma_start(out=bv_sb[:], in_=bv_d[:])

            # ---- Q: load qT, project ----
            qT = qT_pool.tile([128, HC * QSH], bf16)
            nc.sync.dma_start(
                out=qT[:].rearrange("p (c l) -> p c l", c=HC),
                in_=qT_d.rearrange("(c p) l -> p c l", p=128),
            )
            qprojT = proj_pool.tile([D, QSH], bf16, tag="qprojT")
            for qn in range(QSH // 512):
                ps = pj_psum.tile([D, 512], f32, tag="pj", name="pjq")
                for hc in range(HC):
                    nc.tensor.matmul(
                        ps[:],
                        w_sb["wq"][:, hc * D : (hc + 1) * D],
                        qT[:, hc * QSH + qn * 512 : hc * QSH + (qn + 1) * 512],
                        start=(hc == 0), stop=(hc == HC - 1),
                    )
                nc.scalar.activation(
                    qprojT[:, qn * 512 : (qn + 1) * 512], ps[:],
                    AF.Identity, bias=bq_sb[:], scale=0.125,
                )

            # ---- V: load vT per range, project, build vp (natural + ones) ----
            vp = vp_pool.tile([128, (L // 128) * 65], bf16, tag="vp")
            for rng in range(NRNG):
                vt = kv_pool.tile([128, HC * 512], bf16, tag="kv",
                                  name=f"vT{rng}")
                nc.scalar.dma_start(
                    out=vt[:].rearrange("p (c l) -> p c l", c=HC),
                    in_=vT_d.rearrange("(c p) l -> p c l", p=128)[
                        :, :, rng * 512 : (rng + 1) * 512
                    ],
                )
                ps = pj_psum.tile([D, 512], f32, tag="pj", name="pjv")
                for hc in range(HC):
                    nc.tensor.matmul(
                        ps[:], w_sb["wv"][:, hc * D : (hc + 1) * D],
                        vt[:, hc * 512 : (hc + 1) * 512],
                        start=(hc == 0), stop=(hc == HC - 1),
                    )
                vs = proj_pool.tile([D, 512], bf16, tag=f"vs{rng}",
                                    name=f"vs{rng}")
                nc.scalar.activation(vs[:], ps[:], AF.Identity, bias=bv_sb[:])
                psv = psv_psum.tile([128, 512], bf16, tag="psv", name="psv")
                for s in range(4):
                    nc.tensor.transpose(
                        psv[:, s * 128 : s * 128 + D],
                        vs[:, s * 128 : (s + 1) * 128],
                        identb[0:D, 0:D],
                    )
                for s in range(4):
                    kc = rng * 4 + s
                    nc.vector.tensor_copy(
                        vp[:, kc * 65 : kc * 65 + 64],
                        psv[:, s * 128 : s * 128 + D],
                    )
                    nc.vector.tensor_copy(
                        vp[:, kc * 65 + 64 : kc * 65 + 65], ones_sb[:]
                    )

            # ---- K: load per range, project, scores + attnV ----
            outT_ps = acc_psum.tile([65, QSH], f32)
            for rng in range(NRNG):
                kt = kv_pool.tile([128, HC * 512], bf16, tag="kv",
                                  name=f"kT{rng}")
                nc.sync.dma_start(
                    out=kt[:].rearrange("p (c l) -> p c l", c=HC),
                    in_=kT_d.rearrange("(c p) l -> p c l", p=128)[
                        :, :, rng * 512 : (rng + 1) * 512
                    ],
                )
                ps = pj_psum.tile([D, 512], f32, tag="pj", name="pjk")
                for hc in range(HC):
                    nc.tensor.matmul(
                        ps[:], w_sb["wk"][:, hc * D : (hc + 1) * D],
                        kt[:, hc * 512 : (hc + 1) * 512],
                        start=(hc == 0), stop=(hc == HC - 1),
                    )
                kslice = proj_pool.tile([D, 512], bf16, tag=f"ks{rng}",
                                        name=f"ks{rng}")
                nc.scalar.activation(kslice[:], ps[:], AF.Identity,
                                     bias=bk_sb[:])

                for s in range(4):
                    kc = rng * 4 + s
                    e = exp_pool.tile([128, QSH], bf16, tag="exp")
                    for qn in range(QSH // 512):
                        sc = sc_psum.tile([128, 512], f32, tag="sc")
                        nc.tensor.matmul(
                            sc[:],
                            kslice[:, s * 128 : (s + 1) * 128],
                            qprojT[:, qn * 512 : (qn + 1) * 512],
                            start=True, stop=True,
                        )
                        nc.scalar.activation(
                            e[:, qn * 512 : (qn + 1) * 512], sc[:], AF.Exp
                        )
                    for qn in range(QSH // 512):
                        nc.tensor.matmul(
                            outT_ps[:, qn * 512 : (qn + 1) * 512],
                            vp[:, kc * 65 : (kc + 1) * 65],
                            e[:, qn * 512 : (qn + 1) * 512],
                            start=(kc == 0), stop=(kc == L // 128 - 1),
                            skip_group_check=True,
                        )

            # ---- finalize ----
            outT_sb = fin_pool.tile([65, QSH], f32, tag="outT")
            nc.vector.tensor_copy(outT_sb[:], outT_ps[:])
            out_sb = fin_pool.tile([128, 8 * D], f32, tag="out")
            for qc in range(QSH // 128):
                ps = pj_psum.tile([128, 128], f32, tag="pj", name="pjf")
                nc.tensor.transpose(
                    ps[:, 0:65],
                    outT_sb[:, qc * 128 : (qc + 1) * 128],
                    identf[0:65, 0:65],
                )
                recip = fin_pool.tile([128, 1], f32, tag="recip")
                nc.vector.reciprocal(recip[:], ps[:, 64:65])
                nc.vector.tensor_scalar_mul(
                    out_sb[:, qc * D : (qc + 1) * D], ps[:, 0:D], recip[:]
                )
            nc.sync.dma_start(
                out=out_d.rearrange("(j p) d -> p j d", p=128),
                in_=out_sb[:].rearrange("p (j d) -> p j d", j=8),
            )

    nc.compile()
    return nc


_NC_CACHE = None


def _get_nc():
    global _NC_CACHE
    if _NC_CACHE is None:
        _NC_CACHE = build_bass()
    return _NC_CACHE


def _make_in_maps(inputs):
    import ml_dtypes

    bf = ml_dtypes.bfloat16
    # [N, L, H] -> [N, H, L] transposed bf16, contiguous
    qt = np.ascontiguousarray(
        np.asarray(inputs["query"], np.float32).astype(bf).transpose(0, 2, 1)
    )
    kt = np.ascontiguousarray(
        np.asarray(inputs["key"], np.float32).astype(bf).transpose(0, 2, 1)
    )
    vt = np.ascontiguousarray(
        np.asarray(inputs["value"], np.float32).astype(bf).transpose(0, 2, 1)
    )
    wq = np.ascontiguousarray(np.asarray(inputs["Wq"], np.float32).astype(bf))
    wk = np.ascontiguousarray(np.asarray(inputs["Wk"], np.float32).astype(bf))
    wv = np.ascontiguousarray(np.asarray(inputs["Wv"], np.float32).astype(bf))
    bq8 = (np.asarray(inputs["bq"], np.float32) / 8.0).reshape(D, 1)
    bk = np.asarray(inputs["bk"], np.float32).reshape(D, 1).copy()
    bv = np.asarray(inputs["bv"], np.float32).reshape(D, 1).copy()
    in_maps = []
    for c in range(NCORES):
        b, half = divmod(c, 2)
        in_maps.append(
            {
                "qT": np.ascontiguousarray(
                    qt[b, :, half * QSH : (half + 1) * QSH]
                ),
                "kT": kt[b],
                "vT": vt[b],
                "wq": wq,
                "wk": wk,
                "wv": wv,
                "bq8": bq8,
                "bk": bk,
                "bv": bv,
            }
        )
    return in_maps


def kernel(query, key, value, Wq, bq, Wk, bk, Wv, bv):
    from concourse.bass_utils import run_bass_kernel_spmd

    in_maps = _make_in_maps(
        dict(query=query, key=key, value=value, Wq=Wq, bq=bq, Wk=Wk, bk=bk,
             Wv=Wv, bv=bv)
    )
    nc = _get_nc()
    try:
        res = run_bass_kernel_spmd(nc, in_maps, list(range(NCORES)))
    except Exception:
        res = run_bass_kernel_spmd(nc, in_maps, list(range(NCORES)))
    out = np.empty((N, L, D), np.float32)
    for c in range(NCORES):
        b, half = divmod(c, 2)
        out[b, half * QSH : (half + 1) * QSH] = res.results[c]["out"]
    return out
